# revision 61
# baseline (speedup 1.0000x reference)
"""Trainium2 Bass kernel for DFMN prototypical-network loss (retrieval_knn).

Reference math (per pixel, 64-way episode):
    protos = prototypes[indices]                         # [64, C]
    logits = -(|q|^2 + |p|^2 - 2 q.p)                    # [N, 64]
    loss   = -mean(log_softmax(logits)[label])

Key identity: the per-pixel |q|^2 term is constant across classes, so it
cancels in log_softmax.  With G = q.p and z = 2G - pn (pn = |p|^2 per class):
    -logp[label] = LSE_c(z) - z[label]
    loss = [ sum_px LSE_c(z) - sum_img (2*rowsumG[label_b] - 196*pn[label_b]) ] / N

Device layout per core (64 images, data-parallel over 8 cores).  Work is
organized in 16 "units" of 2 image pairs; the two pairs of a unit occupy the
two partition halves of one PSUM bank via PE column tiling (tile_position
(0,0) / (0,64)), so the exp and reductions run on full 128-partition tiles:
  - G via TensorE:   G[128, 392] = protosT_k.T @ Q_k  (8 K-chunks, fp8e4,
    col-tiled pair of matmuls per unit; prototype weights loaded once per
    K-chunk per group and redundant LDWEIGHTS deduplicated)
  - exp via ScalarE: e = Exp(2*G - pn)  (PSUM -> SBUF bf16, fused scale/bias)
  - colsum via TensorE: s[32, 392] += selector.T @ e  (selector has a ones
    column per partition half; one accumulation chain over units 0..12)
  - label rowsums via VectorE: r2[128, 26] = per-image free-dim reduce of G
  - final ScalarE Ln with accum_out -> per-row sums of log s, written
    into column 26 of the rsum output (no separate tiny lse DMA)
  - units 13..15 ship raw e to the host (etail), which computes both their
    LSE rows and their label terms (log e = 2G - pn) in float64 — no
    selector / Ln / reduce work trails the q stream on device.
Host finishes: label gather from r2, pn terms, exact float64 mean.

Streaming dtype is fp8e4 (TRN e4m3, max +-240): quarters the fp32 HBM
traffic (12.85 MB/core at ~358 GB/s is the roofline) and quantization error
lands ~5e-4 relative on the loss (gate is 2e-2).  The q DRAM layout is
partition-major [128, NPAIR*QCOLS] so every group DMA is one contiguous run
per partition (128 fat descriptors instead of ~770 thin ones).

Both Exp and Ln resolve to the `natural_log_exp_and_others` activation
table set (it contains both), so the kernel performs exactly one
ACT_TABLE_LOAD — the baseline paid 4 switches of ~1.3us, one on the tail.

PE HAM management: the tensor engine's clock is gated 4/8 (1.2 GHz) by
default and only reaches 2.4 GHz after ~3.4us of sustained busy (PE_HAM
activity window); a ~3.4us idle re-throttles it.  At the warm clock the
col-tiled fp8 matmul stream runs ~2 cols/cycle aggregate (~83 ns/matmul
sustained), which makes the kernel q-stream-bound, so the whole game is
keeping the PE warm: an 11-matmul zero accumulation chain into a dead
PSUM bank (gapless — the HAM needs a complete window of uninterrupted
busy) warms the clock across the preamble + first-DMA latency, and
trickle matmuls gated on the head units' exps split the early stream
waits so no full HAM window goes idle.  qpool holds 5 group buffers so
the q stream never throttles on PSUM/PE backpressure even when the head
units run cold (the PE's real work is only ~25us of the ~40us stream).
"""

import sys

for _p in ("/opt/trn_rl_repo",):
    if _p not in sys.path:
        sys.path.insert(0, _p)

import numpy as np

import bass_rust as _bass_rust
import concourse.bass as bass
import concourse.bacc as bacc
import concourse.tile as tile
from concourse import mybir
from concourse.bass_utils import run_bass_kernel_spmd
from concourse.hw_specs import get_activation_tables

# Problem constants (nn_DFMNLoss: B=512, C=1024, 14x14 features, 64-way)
B = 512
C = 1024
F2 = 196          # 14 * 14 pixels per image
NWAY = 64
NCORES = 8
BPC = B // NCORES           # 64 images per core
NPAIR = BPC // 2            # 32 image pairs per core
NU = NPAIR // 2             # 16 col-tiled units (2 pairs each)
KT = C // 128               # 8 contraction chunks of 128 channels
PAIRCOLS = 2 * F2           # 392 pixel columns per pair
QCOLS = KT * PAIRCOLS       # 3136 fp8 elements per partition per pair
HK = (KT // 2) * PAIRCOLS   # 1568: half-k block of a pair

# Group plan: leading singleton starts the PE early (its halves are
# 0.4 MB); 2-unit groups mid-stream; the tail tapers to singletons so the
# PE's backlog when the (roofline-bound) q stream ends is at most one
# unit's matmuls, not a whole 1.6 MB group.
PLAN = [[0], [1], [2, 3], [4, 5], [6, 7], [8, 9],
        [10, 11], [12, 13], [14], [15]]
CHAIN_A_LAST = 12           # selector chain covers units 0..12; units
NETAIL = 3                  # 13,14,15 ship raw e to the host instead

F32 = mybir.dt.float32
F16 = mybir.dt.float16
BF16 = mybir.dt.bfloat16
F8 = mybir.dt.float8e4
F8_NP = mybir.dt.np(F8)
BF16_NP = mybir.dt.np(BF16)

_CACHE = {}


class _Bacc(bacc.Bacc):
    def insert_act_table_loads(self):
        """Same pass as Bacc, but Exp/Ln are removed from every activation
        table set except `natural_log_exp_and_others` (which genuinely
        contains both), so the fixpoint places a single load for the whole
        kernel instead of thrashing exp_and_others <-> natural_log."""
        has_activation = any(
            isinstance(i, mybir.InstActivation)
            for b in self.main_func.blocks
            for i in b.instructions
        )
        if not has_activation:
            return
        exp_ln = {
            mybir.ActivationFunctionType.Exp,
            mybir.ActivationFunctionType.Ln,
            mybir.ActivationFunctionType.Copy,
        }
        tables = [
            (name, fns if name == "natural_log_exp_and_others" else fns - exp_ln)
            for name, fns in get_activation_tables(self.m.arch).items()
        ]
        _bass_rust.insert_act_table_loads(self, tables)


def _strip_keys(inst):
    """Per-32-column-strip content keys for an LDW: strip cg of the PE array
    holds source elements starting at offset + (32*cg - tile_col).  Lets a
    64-wide reload be recognized as redundant against a previous 128-wide
    load of duplicated weights.  Falls back to an opaque whole-AP key when
    the AP isn't a contiguous-column 2D weights load."""
    a = inst.ins[0]
    tp = inst.tile_position or (0, 0)
    ts = inst.tile_size or (128, 128)
    cgs = list(range(tp[1] // 32, (tp[1] + ts[1] + 31) // 32))
    ap = a.ap
    if len(ap) == 2 and ap[1][0] == 1 and tp[0] == 0:
        return {
            cg: (a.memref, a.offset + 32 * cg - tp[1], str(a.dtype), str(ap[0]))
            for cg in cgs
        }
    key = (a.memref, a.offset, str(a.ap), str(a.dtype), tp, ts)
    return {cg: key for cg in cgs}


def _dedup_ldweights(nc):
    """Drop InstLdweights that reload weights already resident in the PE
    array.  Tile emits one LDWEIGHTS per matmul; inside a group the same
    prototype chunk is loaded for every unit, and the loads serialize with
    the matmul stream on the PE.  Matmults here are non-self-loading
    (ldweights=False), so a dropped reload just reuses the array contents.

    (Widening the lead 64-wide load to cover both column halves was tried
    and cost ~50ns per matmul: the per-half loads let the next K-chunk's
    weights stream into one half of the array while the other half's
    matmul still drains; a single 128-wide load serializes that.)

    Tracks per-strip CONTENT keys per 32-wide PE column group (carried
    across basic blocks — this program is a pure fallthrough chain); only
    sync-free LDWs are dropped, and any dangling dependency names are
    remapped to the keeper.
    """
    removed = {}
    state = {}  # col_group -> (key, keeper_name)
    for blk in nc.m.functions[0].blocks:
        kept = []
        for inst in blk.instructions:
            if isinstance(inst, mybir.InstLdweights):
                keys = _strip_keys(inst)
                si = inst.sync_info
                clean = si is None or (not si.on_wait and not si.on_update)
                prev = [state.get(cg) for cg in keys]
                if clean and all(
                    p is not None and p[0] == k
                    for p, k in zip(prev, keys.values())
                ):
                    removed[inst.name] = prev[0][1]
                    continue
                for cg, k in keys.items():
                    state[cg] = (k, inst.name)
            kept.append(inst)
        blk.instructions[:] = kept
    if removed:
        for blk in nc.m.functions[0].blocks:
            for inst in blk.instructions:
                names = set(inst.sync_dependency_names()) | set(
                    inst.nosync_dependency_names()
                )
                if names & removed.keys():
                    inst.remap_dependency_names(
                        {k: v for k, v in removed.items() if k in names}
                    )
        for k in removed:
            nc.inst_map.pop(k, None)
    return len(removed)


def _build_nc():
    # Bacc (not raw Bass): its compile() pass splits multi-wait instructions
    # into event semaphores — walrus allows only one sync wait per instruction.
    nc = _Bacc()
    q = nc.dram_tensor("q", [128, NPAIR * QCOLS], F8, kind="ExternalInput")
    pT = nc.dram_tensor("pT", [128, KT * NWAY], F8, kind="ExternalInput")
    negpn2 = nc.dram_tensor("negpn2", [128, 1], F32, kind="ExternalInput")
    bsel2 = nc.dram_tensor("bsel2", [128, 2 * NPAIR - 1], BF16, kind="ExternalInput")
    # rsum[:, 0:26] = per-image G rowsums of units 0..12;
    # rsum[0:26, 26] = chain-A Ln accums (their lse total).
    rsum = nc.dram_tensor(
        "rsum", [128, 2 * (CHAIN_A_LAST + 1) + 1], F32, kind="ExternalOutput"
    )
    # The tail units' exps ship raw: the host does their LSE rows (colsum +
    # log in float64) AND their label rowsums (log e recovers 2G - pn), so
    # no selector / Ln / reduce work for them trails the q stream on device.
    etail = nc.dram_tensor(
        "etail", [128, NETAIL * PAIRCOLS], BF16, kind="ExternalOutput"
    )

    NRED = 2 * (CHAIN_A_LAST + 1)   # 26 rowsum columns kept on device

    with tile.TileContext(nc) as tc:
        with (
            tc.tile_pool(name="const", bufs=1) as cpool,
            tc.tile_pool(name="qin", bufs=5) as qpool,
            tc.tile_pool(name="qtail", bufs=1) as tpool,
            tc.tile_pool(name="acc", bufs=1) as apool,
            tc.tile_pool(name="gps", bufs=6, space="PSUM") as gpool,
            tc.tile_pool(name="sps", bufs=1, space="PSUM") as spool,
            tc.tile_pool(name="wps", bufs=1, space="PSUM") as wpool,
        ):
            # Singleton groups stream in per-pair halves/quarters so the
            # first matmul waits on 0.4 MB, not a whole group.  The constants
            # go through the ACT HW-DGE ring (nc.scalar) so they drain
            # immediately instead of queueing behind the q stream on the SP
            # ring — the first LDWEIGHTS needs pT.  (Bulk q data must never
            # ride the scalar ring: once the SP-ring stream is running, the
            # DMA engines round-robin 1:1 between the queues' descriptors, so
            # a second queue drains ~8x slow AND steals ~13% of the stream.)
            #
            # Singletons run per-pair k-chains (pair a fully, then pair b),
            # so the head singleton needs only 2 fat DMAs (each pair is
            # contiguous in DRAM) — descriptor generation on the sync
            # sequencer costs ~640ns per DMA and directly delays the group
            # stream behind it.  The tail singleton keeps 4 quarter DMAs
            # ordered a1,a2,b1,b2 so only pair-b's k4..7 matmuls and the
            # b-half exp trail the final q byte.
            parts = {}

            def head_dma(u):
                # ONE DMA for the whole head singleton (pairs are adjacent
                # in DRAM): descriptor generation is serial on the sync
                # sequencer at ~0.64us per DMA and directly delays every
                # stream DMA queued behind it, while the PE's first k-chain
                # is gated by the HAM warmup chain (~12.3us), not by data
                # arrival (~10.5us) — the old per-pair split bought nothing.
                # (Routing any head pair via the scalar ring was tried twice
                # and costs +5-8us: once the SP-ring stream runs, a second
                # queue drains ~8x slow and the in-order PE blocks on it.)
                # (Folding unit 1 into this DMA as well was tried and cost
                # +8us — both units' chains then gate on one 1.6 MB arrival
                # and the in-order PE serializes behind it.)
                pa = 2 * u
                t = tpool.tile([128, 2 * QCOLS], F8, name="qh", tag=f"qh_{u}")
                nc.sync.dma_start(t[:], q[:, pa * QCOLS : (pa + 2) * QCOLS])
                parts[u] = (
                    [(t, k * PAIRCOLS) for k in range(KT)],
                    [(t, QCOLS + k * PAIRCOLS) for k in range(KT)],
                )

            def tail_dma(u):
                # pair a in two quarters, pair b in one quarter plus two
                # EIGHTHS: only the k6/k7 matmuls (2, not 4) and the b-half
                # exp trail the final q byte.
                pa, pb = 2 * u, 2 * u + 1
                tiles = []
                for nm, pr, k0, w in (
                    ("qa1", pa, 0, HK), ("qa2", pa, HK, HK),
                    ("qb1", pb, 0, HK),
                    ("qb2a", pb, HK, HK // 2), ("qb2b", pb, HK + HK // 2, HK // 2),
                ):
                    t = tpool.tile([128, w], F8, name=nm, tag=f"{nm}_{u}")
                    nc.sync.dma_start(
                        t[:], q[:, pr * QCOLS + k0 : pr * QCOLS + k0 + w]
                    )
                    tiles.append(t)
                ka = [(tiles[0], k * PAIRCOLS) for k in range(4)] + [
                    (tiles[1], (k - 4) * PAIRCOLS) for k in range(4, KT)
                ]
                kb = (
                    [(tiles[2], k * PAIRCOLS) for k in range(4)]
                    + [(tiles[3], (k - 4) * PAIRCOLS) for k in (4, 5)]
                    + [(tiles[4], (k - 6) * PAIRCOLS) for k in (6, 7)]
                )
                parts[u] = (ka, kb)

            p_sb = cpool.tile([128, KT * NWAY], F8)
            nc.scalar.dma_start(p_sb[:], pT[:])
            npn_sb = cpool.tile([128, 1], F32)
            nc.scalar.dma_start(npn_sb[:], negpn2[:])
            bsel_sb = cpool.tile([128, 2 * NPAIR - 1], BF16)
            nc.scalar.dma_start(bsel_sb[:], bsel2[:])

            head_dma(PLAN[0][0])

            r_sb = apool.tile([128, NRED + 1], F32)
            ltmp = apool.tile([NRED, PAIRCOLS], F32)
            # Per-unit e tiles (not one big e_all): with a single tile,
            # Tile's per-tile WAR tracking makes every exp(u) wait for
            # sel(u-1)'s read to finish — a false serialization on ACT —
            # and each such multi-wait instruction costs an event semaphore
            # (the preamble/teardown loops scale with their count).
            e_t = [
                apool.tile([128, PAIRCOLS], BF16, name=f"e{u}", tag=f"e{u}")
                for u in range(NU)
            ]
            sA = spool.tile([NPAIR, PAIRCOLS], F32, name="sA", tag="sA")

            # PE HAM warmup: zero matmuls into a dead PSUM bank.  zq needs no
            # DMA (memset on GpSimd), so the PE goes busy right after its
            # sequencer preamble.  One accumulation chain, NOT independent
            # matmuls: the HAM only un-throttles after a COMPLETE free-running
            # ~3.41us activity window of gapless PE busy-ness (independent
            # same-bank matmuls get sem-chained by Tile, and the ~100ns holes
            # keep every window "not busy").  The chain spans ~4.5us and the
            # head singleton's matmuls follow contiguously, so one full
            # window is covered regardless of the window phase and the PE is
            # at 2.4 GHz before the first group lands.
            zq = cpool.tile([128, 512], F8)
            nc.gpsimd.memset(zq[:], 0)
            warm_ps = wpool.tile([NWAY, 500], F32, name="wps", tag="wps")

            NWARM = 11
            for i in range(NWARM):
                nc.tensor.matmul(
                    warm_ps[:],
                    zq[:, 0:NWAY],
                    zq[:, 0:500],
                    start=(i == 0),
                    stop=(i == NWARM - 1),
                    skip_group_check=True,
                )

            # ACT warmup: absorb the npn DMA wait, const-AP init and the
            # single exp+ln table load outside the hot loop.
            warm_a = cpool.tile([128, 1], F32)
            warm_b = cpool.tile([128, 1], F32)
            nc.scalar.copy(warm_a[:], npn_sb[:])
            nc.scalar.activation(
                warm_b[:], warm_a[:], mybir.ActivationFunctionType.Exp
            )

            def sel_matmul(u):
                # chain A: unit u -> rows 2u, 2u+1 (units 0..12, rows 0..25);
                # row j:  s[j, :] += colsum of the partition half of e(u).
                # units 13..15 have no selector: their e ships via etail.
                if u > CHAIN_A_LAST:
                    return
                j0 = 2 * u
                nc.tensor.matmul(
                    sA[:],
                    bsel_sb[:, NPAIR - 1 - j0 : 2 * NPAIR - 1 - j0],
                    e_t[u][:],
                    start=(u == 0),
                    stop=(u == CHAIN_A_LAST),
                    skip_group_check=True,
                )

            def trickle_mm(u):
                # HAM keep-warm matmul gated on exp(u): runs ~0.5-1.2us into
                # the PE's wait for the next group's DMA, splitting the idle
                # gap so no full ~3.4us HAM window goes idle (which would
                # re-throttle the PE clock to 1.2 GHz for the next ~3.4us).
                nc.tensor.matmul(
                    warm_ps[0:NPAIR, 0:PAIRCOLS],
                    bsel_sb[:, 0:NPAIR],
                    e_t[u][:],
                    start=True,
                    stop=True,
                    skip_group_check=True,
                )

            for gi, units in enumerate(PLAN):
                single = len(units) == 1
                last = gi == len(PLAN) - 1
                gp = 2 * len(units)          # pairs in this group
                p0 = 2 * units[0]            # first pair index
                split_dma = single and (gi == 0 or last)
                if split_dma:
                    if units[0] not in parts:
                        tail_dma(units[0])
                else:
                    # One contiguous-per-partition DMA per group.  (Per-unit
                    # DMA slices into a live tile were tried and slowed the
                    # matmul stream ~1.6x via SBUF write/read contention;
                    # alternating the SP/ACT HW-DGE rings across groups was
                    # also tried and cost ~2.5us — interleaved drain delays
                    # the in-order group completions the PE waits on.)
                    gt = qpool.tile([128, gp * QCOLS], F8, name="gt", tag="gt")
                    nc.sync.dma_start(
                        gt[:], q[:, p0 * QCOLS : (p0 + gp) * QCOLS]
                    )
                gps = {
                    u: gpool.tile([128, PAIRCOLS], F32, name="gps", tag="gps")
                    for u in units
                }
                if split_dma:
                    # per-pair k-chains: pair a (rows 0:64) completes on its
                    # own DMAs, then pair b.
                    u = units[0]
                    for half, segs in enumerate(parts[u]):
                        col0 = half * NWAY
                        for k in range(KT):
                            srct, cc = segs[k]
                            nc.tensor.matmul(
                                gps[u][col0 : col0 + NWAY, :],
                                p_sb[:, k * NWAY : (k + 1) * NWAY],
                                srct[:, cc : cc + PAIRCOLS],
                                tile_position=(0, col0),
                                start=(k == 0),
                                stop=(k == KT - 1),
                                skip_group_check=True,
                            )
                        if last and half == 0:
                            # pair-a chain done; its exp can fire while pair
                            # b still streams, and the whole a-half of etail
                            # ships before the final q byte.
                            nc.scalar.activation(
                                e_t[u][0:NWAY, :],
                                gps[u][0:NWAY, :],
                                mybir.ActivationFunctionType.Exp,
                                bias=npn_sb[0:NWAY, :],
                                scale=2.0,
                            )
                else:
                    # K-outer order inside the group: the two col-halves of
                    # both units share each K-chunk's LDWEIGHTS (redundant
                    # reloads deduplicated below).
                    for k in range(KT):
                        wa = wb = p_sb[:, k * NWAY : (k + 1) * NWAY]
                        for jloc, u in enumerate(units):
                            ca = 2 * jloc * QCOLS + k * PAIRCOLS
                            cb = (2 * jloc + 1) * QCOLS + k * PAIRCOLS
                            nc.tensor.matmul(
                                gps[u][0:NWAY, :],
                                wa,
                                gt[:, ca : ca + PAIRCOLS],
                                tile_position=(0, 0),
                                start=(k == 0),
                                stop=(k == KT - 1),
                                skip_group_check=True,
                            )
                            nc.tensor.matmul(
                                gps[u][NWAY:128, :],
                                wb,
                                gt[:, cb : cb + PAIRCOLS],
                                tile_position=(0, NWAY),
                                start=(k == 0),
                                stop=(k == KT - 1),
                                skip_group_check=True,
                            )
                # Selector matmuls lag one group so the PE never stalls on
                # the ACT exp (exp(g-1) ran during this group's matmuls).
                if gi > 0:
                    for u in PLAN[gi - 1]:
                        sel_matmul(u)
                    if CHAIN_A_LAST in PLAN[gi - 1]:
                        # chain A closed; its Ln (and the rowsum output
                        # behind it on the SP ring) runs during the tail
                        # units' streaming, off the critical path.
                        nc.scalar.activation(
                            ltmp[:],
                            sA[0:NRED, :],
                            mybir.ActivationFunctionType.Ln,
                            accum_out=r_sb[0:NRED, NRED : NRED + 1],
                        )
                for u in units:
                    if last:
                        # only the b-half exp trails the final q byte (the
                        # a-half ran inside the split k-chain above).
                        nc.scalar.activation(
                            e_t[u][NWAY:128, :],
                            gps[u][NWAY:128, :],
                            mybir.ActivationFunctionType.Exp,
                            bias=npn_sb[NWAY:128, :],
                            scale=2.0,
                        )
                        continue
                    nc.scalar.activation(
                        e_t[u][:],
                        gps[u][:],
                        mybir.ActivationFunctionType.Exp,
                        bias=npn_sb[:],
                        scale=2.0,
                    )
                    if u <= CHAIN_A_LAST:
                        # one reduce for both images of the unit: X reduces
                        # the innermost (pixel) dim of the [128, 2, 196] view
                        nc.vector.reduce_sum(
                            r_sb[:, 2 * u : 2 * u + 2],
                            gps[u][:].rearrange("p (i f) -> p i f", i=2),
                            axis=mybir.AxisListType.X,
                        )
                    # Trickles only at the head, where the PE runs well
                    # ahead of the stream (gaps up to ~3us); mid-stream
                    # trickles were tried (u<=9) and cost ~2us — each one
                    # delays the next unit's matmuls behind its exp(u) wait.
                    # At the tail the PE is the critical path outright.
                    if u <= 1:
                        trickle_mm(u)
            # Outputs.  e13/e14/rsum queue on the SP ring BEHIND the q
            # stream (in-queue order, so they cannot steal DMA-engine slots
            # from it mid-stream; their DGE waits resolve before the queue
            # reaches them).  The tail unit's two e halves go on the
            # (otherwise empty) scalar ring: the a-half fires as soon as
            # quarter qa2's chain+exp finish, and the b-half — the kernel's
            # last output — right behind it.  (Routing them on the SP ring
            # after rsum measured no better.)
            nc.scalar.dma_start(
                etail[0:NWAY, (NETAIL - 1) * PAIRCOLS : NETAIL * PAIRCOLS],
                e_t[NU - 1][0:NWAY, :],
            )
            nc.scalar.dma_start(
                etail[NWAY:128, (NETAIL - 1) * PAIRCOLS : NETAIL * PAIRCOLS],
                e_t[NU - 1][NWAY:128, :],
            )
            for t in range(NETAIL - 1):
                u = NU - NETAIL + t
                nc.sync.dma_start(
                    etail[:, t * PAIRCOLS : (t + 1) * PAIRCOLS],
                    e_t[u][:],
                )
            nc.sync.dma_start(rsum[:], r_sb[:])

    n = _dedup_ldweights(nc)
    if n < 64:
        print(f"[kernel] warning: ldweights dedup removed only {n}", flush=True)
    nc.compile()
    return nc


def _get_nc():
    if "nc" not in _CACHE:
        _CACHE["nc"] = _build_nc()
    return _CACHE["nc"]


def _pack_core_q(qc8):
    # fp8 [64, C, F2] -> [p, pair, k, i, f] -> [128, NPAIR*QCOLS]
    qc = qc8.reshape(NPAIR, 2, KT, 128, F2).transpose(3, 0, 2, 1, 4)
    return np.ascontiguousarray(qc).reshape(128, NPAIR * QCOLS)


def _prepare(query_features, labels, prototypes, indices):
    """Returns (in_maps, labels_i64, pn64)."""
    qf = np.asarray(query_features, dtype=np.float32).reshape(B, C, F2)
    labels = np.asarray(labels).astype(np.int64)
    protos = np.asarray(prototypes, dtype=np.float32)
    idx = np.asarray(indices).astype(np.int64)

    pg8 = protos[idx].astype(F8_NP)                      # [64, C] fp8
    pg = pg8.astype(np.float64)
    pn64 = np.sum(pg**2, axis=1)                         # matches device G
    negpn2_np = np.ascontiguousarray(
        np.concatenate([-pn64, -pn64]).reshape(128, 1).astype(np.float32)
    )
    pT_pack = np.ascontiguousarray(
        pg8.T.reshape(KT, 128, NWAY).transpose(1, 0, 2)
    ).reshape(128, KT * NWAY)
    bsel2_np = np.zeros((128, 2 * NPAIR - 1), dtype=BF16_NP)
    bsel2_np[0:NWAY, NPAIR - 1] = 1
    bsel2_np[NWAY:128, NPAIR] = 1

    qf8 = qf.astype(F8_NP)
    in_maps = [
        {
            "q": _pack_core_q(qf8[c * BPC : (c + 1) * BPC]),
            "pT": pT_pack,
            "negpn2": negpn2_np,
            "bsel2": bsel2_np,
        }
        for c in range(NCORES)
    ]
    return in_maps, labels, pn64


def kernel(query_features, labels, prototypes, indices, n_way):
    import time as _time

    t0 = _time.time()
    nc = _get_nc()
    t1 = _time.time()
    in_maps, labels, pn64 = _prepare(query_features, labels, prototypes, indices)
    t2 = _time.time()
    results = run_bass_kernel_spmd(nc, in_maps, list(range(NCORES))).results
    t3 = _time.time()
    print(
        f"[kernel] build={t1 - t0:.1f}s pack={t2 - t1:.1f}s run={t3 - t2:.1f}s",
        flush=True,
    )

    # Host-side finish: rsum[:, 0:26] holds per-image rowsums of G for units
    # 0..12; image local index l lives at row block 64*(l%4>=2)+class,
    # column 2*(l//4)+(l%2).  rsum[0:26, 26] holds the chain-A per-pair-row
    # sums of log s.  Units 13..15 ship raw e = exp(2G - pn) in etail: the
    # host does both their LSE (colsum+log) and their label terms, using
    # log e[label_row] = 2*G[label_row] - pn[label] summed over the image's
    # pixel columns — exactly the per-image label term.
    NRED = 2 * (CHAIN_A_LAST + 1)
    ndev = NRED * 2                                      # images via device rowsums
    larr = np.arange(ndev)
    rows0 = 64 * ((larr % 4) >= 2)
    cols = 2 * (larr // 4) + (larr % 2)
    total_lse = 0.0
    label_term = 0.0
    for c in range(NCORES):
        out = results[c]["rsum"].astype(np.float64)      # [128, 27]
        total_lse += float(out[0:NRED, NRED].sum())
        et = results[c]["etail"].astype(np.float64)      # [128, 3*392]
        total_lse += float(
            np.log(et[0:NWAY].sum(axis=0)).sum()
            + np.log(et[NWAY:128].sum(axis=0)).sum()
        )
        lab = labels[c * BPC : (c + 1) * BPC]
        r2 = out[:, 0:NRED]
        label_term += float(
            np.sum(2.0 * r2[rows0 + lab[:ndev], cols] - F2 * pn64[lab[:ndev]])
        )
        for l in range(ndev, BPC):
            u = l // 4
            blk = et[:, (u - (NU - NETAIL)) * PAIRCOLS + (l % 2) * F2 :
                     (u - (NU - NETAIL)) * PAIRCOLS + (l % 2) * F2 + F2]
            row = 64 * ((l % 4) >= 2) + lab[l]
            label_term += float(np.log(blk[row]).sum())
    loss = (total_lse - label_term) / (B * F2)
    return np.asarray(loss, dtype=np.float32)



# revision 65
# speedup vs baseline: 1.0202x; 1.0202x over previous
"""Trainium2 Bass kernel for DFMN prototypical-network loss (retrieval_knn).

Reference math (per pixel, 64-way episode):
    protos = prototypes[indices]                         # [64, C]
    logits = -(|q|^2 + |p|^2 - 2 q.p)                    # [N, 64]
    loss   = -mean(log_softmax(logits)[label])

Key identity: the per-pixel |q|^2 term is constant across classes, so it
cancels in log_softmax.  With G = q.p and z = 2G - pn (pn = |p|^2 per class):
    -logp[label] = LSE_c(z) - z[label]
    loss = [ sum_px LSE_c(z) - sum_img (2*rowsumG[label_b] - 196*pn[label_b]) ] / N

Device layout per core (64 images, data-parallel over 8 cores).  Work is
organized in 16 "units" of 2 image pairs; the two pairs of a unit occupy the
two partition halves of one PSUM bank via PE column tiling (tile_position
(0,0) / (0,64)), so the exp and reductions run on full 128-partition tiles:
  - G via TensorE:   G[128, 392] = protosT_k.T @ Q_k  (8 K-chunks, fp8e4,
    col-tiled pair of matmuls per unit; prototype weights loaded once per
    K-chunk per group and redundant LDWEIGHTS deduplicated)
  - exp via ScalarE: e = Exp(2*G - pn)  (PSUM -> SBUF bf16, fused scale/bias)
  - colsum via TensorE: s[32, 392] += selector.T @ e  (selector has a ones
    column per partition half; one accumulation chain over units 0..12)
  - label rowsums via VectorE: r2[128, 26] = per-image free-dim reduce of G
  - final ScalarE Ln with accum_out -> per-row sums of log s, written
    into column 26 of the rsum output (no separate tiny lse DMA)
  - units 13..15 ship raw e to the host (etail), which computes both their
    LSE rows and their label terms (log e = 2G - pn) in float64 — no
    selector / Ln / reduce work trails the q stream on device.
Host finishes: label gather from r2, pn terms, exact float64 mean.

Streaming dtype is fp8e4 (TRN e4m3, max +-240): quarters the fp32 HBM
traffic (12.85 MB/core at ~358 GB/s is the roofline) and quantization error
lands ~5e-4 relative on the loss (gate is 2e-2).  The q DRAM layout is
partition-major [128, NPAIR*QCOLS] so every group DMA is one contiguous run
per partition (128 fat descriptors instead of ~770 thin ones).

Both Exp and Ln resolve to the `natural_log_exp_and_others` activation
table set (it contains both), so the kernel performs exactly one
ACT_TABLE_LOAD — the baseline paid 4 switches of ~1.3us, one on the tail.

PE HAM management: the tensor engine's clock is gated 4/8 (1.2 GHz) by
default and only reaches 2.4 GHz after ~3.4us of sustained busy (PE_HAM
activity window); a ~3.4us idle re-throttles it.  At the warm clock the
col-tiled fp8 matmul stream runs ~2 cols/cycle aggregate (~83 ns/matmul
sustained), which makes the kernel q-stream-bound, so the whole game is
keeping the PE warm: an 11-matmul zero accumulation chain into a dead
PSUM bank (gapless — the HAM needs a complete window of uninterrupted
busy) warms the clock across the preamble + first-DMA latency, and
trickle matmuls gated on the head units' exps split the early stream
waits so no full HAM window goes idle.  qpool holds 5 group buffers so
the q stream never throttles on PSUM/PE backpressure even when the head
units run cold (the PE's real work is only ~25us of the ~40us stream).
"""

import sys

for _p in ("/opt/trn_rl_repo",):
    if _p not in sys.path:
        sys.path.insert(0, _p)

import numpy as np

import bass_rust as _bass_rust
import concourse.bass as bass
import concourse.bacc as bacc
import concourse.tile as tile
from concourse import mybir
from concourse.bass_utils import run_bass_kernel_spmd
from concourse.hw_specs import get_activation_tables

# Problem constants (nn_DFMNLoss: B=512, C=1024, 14x14 features, 64-way)
B = 512
C = 1024
F2 = 196          # 14 * 14 pixels per image
NWAY = 64
NCORES = 8
BPC = B // NCORES           # 64 images per core
NPAIR = BPC // 2            # 32 image pairs per core
NU = NPAIR // 2             # 16 col-tiled units (2 pairs each)
KT = C // 128               # 8 contraction chunks of 128 channels
PAIRCOLS = 2 * F2           # 392 pixel columns per pair
QCOLS = KT * PAIRCOLS       # 3136 fp8 elements per partition per pair
HK = (KT // 2) * PAIRCOLS   # 1568: half-k block of a pair

# Group plan: merged head singleton, 2-unit groups while the stream ramps,
# then singles from unit 6 on — each arrival gap stays well inside the
# ~3.4us HAM window through the contention-dip and tail regions, and the
# PE's backlog at stream end is at most one unit's matmuls.  (All-singles
# was tried and costs +5.5us: the two extra head-region DIRECT2D
# generations — ~0.64us each, serial on the sync sequencer — delay the
# whole stream while it is still descriptor-supply-limited.)
PLAN = [[0], [1], [2, 3], [4, 5], [6], [7], [8], [9],
        [10], [11], [12], [13], [14], [15]]
CHAIN_A_LAST = 12           # selector chain covers units 0..12; units
NETAIL = 3                  # 13,14,15 ship raw e to the host instead

F32 = mybir.dt.float32
F16 = mybir.dt.float16
BF16 = mybir.dt.bfloat16
F8 = mybir.dt.float8e4
F8_NP = mybir.dt.np(F8)
BF16_NP = mybir.dt.np(BF16)

_CACHE = {}


class _Bacc(bacc.Bacc):
    def insert_act_table_loads(self):
        """Same pass as Bacc, but Exp/Ln are removed from every activation
        table set except `natural_log_exp_and_others` (which genuinely
        contains both), so the fixpoint places a single load for the whole
        kernel instead of thrashing exp_and_others <-> natural_log."""
        has_activation = any(
            isinstance(i, mybir.InstActivation)
            for b in self.main_func.blocks
            for i in b.instructions
        )
        if not has_activation:
            return
        exp_ln = {
            mybir.ActivationFunctionType.Exp,
            mybir.ActivationFunctionType.Ln,
            mybir.ActivationFunctionType.Copy,
        }
        tables = [
            (name, fns if name == "natural_log_exp_and_others" else fns - exp_ln)
            for name, fns in get_activation_tables(self.m.arch).items()
        ]
        _bass_rust.insert_act_table_loads(self, tables)


def _strip_keys(inst):
    """Per-32-column-strip content keys for an LDW: strip cg of the PE array
    holds source elements starting at offset + (32*cg - tile_col).  Lets a
    64-wide reload be recognized as redundant against a previous 128-wide
    load of duplicated weights.  Falls back to an opaque whole-AP key when
    the AP isn't a contiguous-column 2D weights load."""
    a = inst.ins[0]
    tp = inst.tile_position or (0, 0)
    ts = inst.tile_size or (128, 128)
    cgs = list(range(tp[1] // 32, (tp[1] + ts[1] + 31) // 32))
    ap = a.ap
    if len(ap) == 2 and ap[1][0] == 1 and tp[0] == 0:
        return {
            cg: (a.memref, a.offset + 32 * cg - tp[1], str(a.dtype), str(ap[0]))
            for cg in cgs
        }
    key = (a.memref, a.offset, str(a.ap), str(a.dtype), tp, ts)
    return {cg: key for cg in cgs}


def _dedup_ldweights(nc):
    """Drop InstLdweights that reload weights already resident in the PE
    array.  Tile emits one LDWEIGHTS per matmul; inside a group the same
    prototype chunk is loaded for every unit, and the loads serialize with
    the matmul stream on the PE.  Matmults here are non-self-loading
    (ldweights=False), so a dropped reload just reuses the array contents.

    (Widening the lead 64-wide load to cover both column halves was tried
    and cost ~50ns per matmul: the per-half loads let the next K-chunk's
    weights stream into one half of the array while the other half's
    matmul still drains; a single 128-wide load serializes that.)

    Tracks per-strip CONTENT keys per 32-wide PE column group (carried
    across basic blocks — this program is a pure fallthrough chain); only
    sync-free LDWs are dropped, and any dangling dependency names are
    remapped to the keeper.
    """
    removed = {}
    state = {}  # col_group -> (key, keeper_name)
    for blk in nc.m.functions[0].blocks:
        kept = []
        for inst in blk.instructions:
            if isinstance(inst, mybir.InstLdweights):
                keys = _strip_keys(inst)
                si = inst.sync_info
                clean = si is None or (not si.on_wait and not si.on_update)
                prev = [state.get(cg) for cg in keys]
                if clean and all(
                    p is not None and p[0] == k
                    for p, k in zip(prev, keys.values())
                ):
                    removed[inst.name] = prev[0][1]
                    continue
                for cg, k in keys.items():
                    state[cg] = (k, inst.name)
            kept.append(inst)
        blk.instructions[:] = kept
    if removed:
        for blk in nc.m.functions[0].blocks:
            for inst in blk.instructions:
                names = set(inst.sync_dependency_names()) | set(
                    inst.nosync_dependency_names()
                )
                if names & removed.keys():
                    inst.remap_dependency_names(
                        {k: v for k, v in removed.items() if k in names}
                    )
        for k in removed:
            nc.inst_map.pop(k, None)
    return len(removed)


def _build_nc():
    # Bacc (not raw Bass): its compile() pass splits multi-wait instructions
    # into event semaphores — walrus allows only one sync wait per instruction.
    nc = _Bacc()
    q = nc.dram_tensor("q", [128, NPAIR * QCOLS], F8, kind="ExternalInput")
    pT = nc.dram_tensor("pT", [128, KT * NWAY], F8, kind="ExternalInput")
    negpn2 = nc.dram_tensor("negpn2", [128, 1], F32, kind="ExternalInput")
    bsel2 = nc.dram_tensor("bsel2", [128, 2 * NPAIR - 1], BF16, kind="ExternalInput")
    # rsum[:, 0:26] = per-image G rowsums of units 0..12;
    # rsum[0:26, 26] = chain-A Ln accums (their lse total).
    rsum = nc.dram_tensor(
        "rsum", [128, 2 * (CHAIN_A_LAST + 1) + 1], F32, kind="ExternalOutput"
    )
    # The tail units' exps ship raw: the host does their LSE rows (colsum +
    # log in float64) AND their label rowsums (log e recovers 2G - pn), so
    # no selector / Ln / reduce work for them trails the q stream on device.
    etail = nc.dram_tensor(
        "etail", [128, NETAIL * PAIRCOLS], BF16, kind="ExternalOutput"
    )

    NRED = 2 * (CHAIN_A_LAST + 1)   # 26 rowsum columns kept on device

    with tile.TileContext(nc) as tc:
        with (
            tc.tile_pool(name="const", bufs=1) as cpool,
            tc.tile_pool(name="qin", bufs=5) as qpool,
            tc.tile_pool(name="qtail", bufs=1) as tpool,
            tc.tile_pool(name="acc", bufs=1) as apool,
            tc.tile_pool(name="gps", bufs=6, space="PSUM") as gpool,
            tc.tile_pool(name="sps", bufs=1, space="PSUM") as spool,
            tc.tile_pool(name="wps", bufs=1, space="PSUM") as wpool,
        ):
            # Singleton groups stream in per-pair halves/quarters so the
            # first matmul waits on 0.4 MB, not a whole group.  The constants
            # go through the ACT HW-DGE ring (nc.scalar) so they drain
            # immediately instead of queueing behind the q stream on the SP
            # ring — the first LDWEIGHTS needs pT.  (Bulk q data must never
            # ride the scalar ring: once the SP-ring stream is running, the
            # DMA engines round-robin 1:1 between the queues' descriptors, so
            # a second queue drains ~8x slow AND steals ~13% of the stream.)
            #
            # Singletons run per-pair k-chains (pair a fully, then pair b),
            # so the head singleton needs only 2 fat DMAs (each pair is
            # contiguous in DRAM) — descriptor generation on the sync
            # sequencer costs ~640ns per DMA and directly delays the group
            # stream behind it.  The tail singleton keeps 4 quarter DMAs
            # ordered a1,a2,b1,b2 so only pair-b's k4..7 matmuls and the
            # b-half exp trail the final q byte.
            parts = {}

            def head_dma(u):
                # ONE DMA for the whole head singleton (pairs are adjacent
                # in DRAM): descriptor generation is serial on the sync
                # sequencer at ~0.64us per DMA and directly delays every
                # stream DMA queued behind it, while the PE's first k-chain
                # is gated by the HAM warmup chain (~12.3us), not by data
                # arrival (~10.5us) — the old per-pair split bought nothing.
                # (Routing any head pair via the scalar ring was tried twice
                # and costs +5-8us: once the SP-ring stream runs, a second
                # queue drains ~8x slow and the in-order PE blocks on it.)
                # (Folding unit 1 into this DMA as well was tried and cost
                # +8us — both units' chains then gate on one 1.6 MB arrival
                # and the in-order PE serializes behind it.)
                pa = 2 * u
                t = tpool.tile([128, 2 * QCOLS], F8, name="qh", tag=f"qh_{u}")
                nc.sync.dma_start(t[:], q[:, pa * QCOLS : (pa + 2) * QCOLS])
                parts[u] = (
                    [(t, k * PAIRCOLS) for k in range(KT)],
                    [(t, QCOLS + k * PAIRCOLS) for k in range(KT)],
                )

            def tail_dma(u):
                # pair a in two quarters, pair b in one quarter plus two
                # EIGHTHS: only the k6/k7 matmuls (2, not 4) and the b-half
                # exp trail the final q byte.
                pa, pb = 2 * u, 2 * u + 1
                tiles = []
                for nm, pr, k0, w in (
                    ("qa1", pa, 0, HK), ("qa2", pa, HK, HK),
                    ("qb1", pb, 0, HK),
                    ("qb2a", pb, HK, HK // 2), ("qb2b", pb, HK + HK // 2, HK // 2),
                ):
                    t = tpool.tile([128, w], F8, name=nm, tag=f"{nm}_{u}")
                    nc.sync.dma_start(
                        t[:], q[:, pr * QCOLS + k0 : pr * QCOLS + k0 + w]
                    )
                    tiles.append(t)
                ka = [(tiles[0], k * PAIRCOLS) for k in range(4)] + [
                    (tiles[1], (k - 4) * PAIRCOLS) for k in range(4, KT)
                ]
                kb = (
                    [(tiles[2], k * PAIRCOLS) for k in range(4)]
                    + [(tiles[3], (k - 4) * PAIRCOLS) for k in (4, 5)]
                    + [(tiles[4], (k - 6) * PAIRCOLS) for k in (6, 7)]
                )
                parts[u] = (ka, kb)

            p_sb = cpool.tile([128, KT * NWAY], F8)
            nc.scalar.dma_start(p_sb[:], pT[:])
            npn_sb = cpool.tile([128, 1], F32)
            nc.scalar.dma_start(npn_sb[:], negpn2[:])
            bsel_sb = cpool.tile([128, 2 * NPAIR - 1], BF16)
            nc.scalar.dma_start(bsel_sb[:], bsel2[:])

            head_dma(PLAN[0][0])

            r_sb = apool.tile([128, NRED + 1], F32)
            ltmp = apool.tile([NRED, PAIRCOLS], F32)
            # Per-unit e tiles (not one big e_all): with a single tile,
            # Tile's per-tile WAR tracking makes every exp(u) wait for
            # sel(u-1)'s read to finish — a false serialization on ACT —
            # and each such multi-wait instruction costs an event semaphore
            # (the preamble/teardown loops scale with their count).
            e_t = [
                apool.tile([128, PAIRCOLS], BF16, name=f"e{u}", tag=f"e{u}")
                for u in range(NU)
            ]
            sA = spool.tile([NPAIR, PAIRCOLS], F32, name="sA", tag="sA")

            # PE HAM warmup: zero matmuls into a dead PSUM bank.  zq needs no
            # DMA (memset on GpSimd), so the PE goes busy right after its
            # sequencer preamble.  One accumulation chain, NOT independent
            # matmuls: the HAM only un-throttles after a COMPLETE free-running
            # ~3.41us activity window of gapless PE busy-ness (independent
            # same-bank matmuls get sem-chained by Tile, and the ~100ns holes
            # keep every window "not busy").  The chain spans ~4.5us and the
            # head singleton's matmuls follow contiguously, so one full
            # window is covered regardless of the window phase and the PE is
            # at 2.4 GHz before the first group lands.
            zq = cpool.tile([128, 512], F8)
            nc.gpsimd.memset(zq[:], 0)
            warm_ps = wpool.tile([NWAY, 500], F32, name="wps", tag="wps")

            NWARM = 11
            for i in range(NWARM):
                nc.tensor.matmul(
                    warm_ps[:],
                    zq[:, 0:NWAY],
                    zq[:, 0:500],
                    start=(i == 0),
                    stop=(i == NWARM - 1),
                    skip_group_check=True,
                )

            # ACT warmup: absorb the npn DMA wait, const-AP init and the
            # single exp+ln table load outside the hot loop.
            warm_a = cpool.tile([128, 1], F32)
            warm_b = cpool.tile([128, 1], F32)
            nc.scalar.copy(warm_a[:], npn_sb[:])
            nc.scalar.activation(
                warm_b[:], warm_a[:], mybir.ActivationFunctionType.Exp
            )

            def sel_matmul(u):
                # chain A: unit u -> rows 2u, 2u+1 (units 0..12, rows 0..25);
                # row j:  s[j, :] += colsum of the partition half of e(u).
                # units 13..15 have no selector: their e ships via etail.
                if u > CHAIN_A_LAST:
                    return
                j0 = 2 * u
                nc.tensor.matmul(
                    sA[:],
                    bsel_sb[:, NPAIR - 1 - j0 : 2 * NPAIR - 1 - j0],
                    e_t[u][:],
                    start=(u == 0),
                    stop=(u == CHAIN_A_LAST),
                    skip_group_check=True,
                )

            def trickle_mm(u):
                # HAM keep-warm matmul gated on exp(u): runs ~0.5-1.2us into
                # the PE's wait for the next group's DMA, splitting the idle
                # gap so no full ~3.4us HAM window goes idle (which would
                # re-throttle the PE clock to 1.2 GHz for the next ~3.4us).
                nc.tensor.matmul(
                    warm_ps[0:NPAIR, 0:PAIRCOLS],
                    bsel_sb[:, 0:NPAIR],
                    e_t[u][:],
                    start=True,
                    stop=True,
                    skip_group_check=True,
                )

            for gi, units in enumerate(PLAN):
                single = len(units) == 1
                last = gi == len(PLAN) - 1
                gp = 2 * len(units)          # pairs in this group
                p0 = 2 * units[0]            # first pair index
                split_dma = single and (gi == 0 or last)
                if split_dma:
                    if units[0] not in parts:
                        tail_dma(units[0])
                else:
                    # One contiguous-per-partition DMA per group.  (Per-unit
                    # DMA slices into a live tile were tried and slowed the
                    # matmul stream ~1.6x via SBUF write/read contention;
                    # alternating the SP/ACT HW-DGE rings across groups was
                    # also tried and cost ~2.5us — interleaved drain delays
                    # the in-order group completions the PE waits on.)
                    gt = qpool.tile([128, gp * QCOLS], F8, name="gt", tag="gt")
                    nc.sync.dma_start(
                        gt[:], q[:, p0 * QCOLS : (p0 + gp) * QCOLS]
                    )
                gps = {
                    u: gpool.tile([128, PAIRCOLS], F32, name="gps", tag="gps")
                    for u in units
                }
                if split_dma:
                    # per-pair k-chains: pair a (rows 0:64) completes on its
                    # own DMAs, then pair b.
                    u = units[0]
                    for half, segs in enumerate(parts[u]):
                        col0 = half * NWAY
                        for k in range(KT):
                            srct, cc = segs[k]
                            nc.tensor.matmul(
                                gps[u][col0 : col0 + NWAY, :],
                                p_sb[:, k * NWAY : (k + 1) * NWAY],
                                srct[:, cc : cc + PAIRCOLS],
                                tile_position=(0, col0),
                                start=(k == 0),
                                stop=(k == KT - 1),
                                skip_group_check=True,
                            )
                        if last and half == 0:
                            # pair-a chain done; its exp can fire while pair
                            # b still streams, and the whole a-half of etail
                            # ships before the final q byte.
                            nc.scalar.activation(
                                e_t[u][0:NWAY, :],
                                gps[u][0:NWAY, :],
                                mybir.ActivationFunctionType.Exp,
                                bias=npn_sb[0:NWAY, :],
                                scale=2.0,
                            )
                else:
                    # K-outer order inside the group: the two col-halves of
                    # both units share each K-chunk's LDWEIGHTS (redundant
                    # reloads deduplicated below).
                    for k in range(KT):
                        wa = wb = p_sb[:, k * NWAY : (k + 1) * NWAY]
                        for jloc, u in enumerate(units):
                            ca = 2 * jloc * QCOLS + k * PAIRCOLS
                            cb = (2 * jloc + 1) * QCOLS + k * PAIRCOLS
                            nc.tensor.matmul(
                                gps[u][0:NWAY, :],
                                wa,
                                gt[:, ca : ca + PAIRCOLS],
                                tile_position=(0, 0),
                                start=(k == 0),
                                stop=(k == KT - 1),
                                skip_group_check=True,
                            )
                            nc.tensor.matmul(
                                gps[u][NWAY:128, :],
                                wb,
                                gt[:, cb : cb + PAIRCOLS],
                                tile_position=(0, NWAY),
                                start=(k == 0),
                                stop=(k == KT - 1),
                                skip_group_check=True,
                            )
                # Selector matmuls lag one group so the PE never stalls on
                # the ACT exp (exp(g-1) ran during this group's matmuls).
                if gi > 0:
                    for u in PLAN[gi - 1]:
                        sel_matmul(u)
                    if CHAIN_A_LAST in PLAN[gi - 1]:
                        # chain A closed; its Ln (and the rowsum output
                        # behind it on the SP ring) runs during the tail
                        # units' streaming, off the critical path.
                        nc.scalar.activation(
                            ltmp[:],
                            sA[0:NRED, :],
                            mybir.ActivationFunctionType.Ln,
                            accum_out=r_sb[0:NRED, NRED : NRED + 1],
                        )
                for u in units:
                    if last:
                        # only the b-half exp trails the final q byte (the
                        # a-half ran inside the split k-chain above).
                        nc.scalar.activation(
                            e_t[u][NWAY:128, :],
                            gps[u][NWAY:128, :],
                            mybir.ActivationFunctionType.Exp,
                            bias=npn_sb[NWAY:128, :],
                            scale=2.0,
                        )
                        continue
                    nc.scalar.activation(
                        e_t[u][:],
                        gps[u][:],
                        mybir.ActivationFunctionType.Exp,
                        bias=npn_sb[:],
                        scale=2.0,
                    )
                    if u <= CHAIN_A_LAST:
                        # one reduce for both images of the unit: X reduces
                        # the innermost (pixel) dim of the [128, 2, 196] view
                        nc.vector.reduce_sum(
                            r_sb[:, 2 * u : 2 * u + 2],
                            gps[u][:].rearrange("p (i f) -> p i f", i=2),
                            axis=mybir.AxisListType.X,
                        )
                    # Trickles only at the head, where the PE runs well
                    # ahead of the stream (gaps up to ~3us); mid-stream
                    # trickles were tried (u<=9) and cost ~2us — each one
                    # delays the next unit's matmuls behind its exp(u) wait.
                    # At the tail the PE is the critical path outright.
                    if u <= 1:
                        trickle_mm(u)
            # Outputs.  e13/e14/rsum queue on the SP ring BEHIND the q
            # stream (in-queue order, so they cannot steal DMA-engine slots
            # from it mid-stream; their DGE waits resolve before the queue
            # reaches them).  The tail unit's two e halves go on the
            # (otherwise empty) scalar ring: the a-half fires as soon as
            # quarter qa2's chain+exp finish, and the b-half — the kernel's
            # last output — right behind it.  (Routing them on the SP ring
            # after rsum measured no better.)
            nc.scalar.dma_start(
                etail[0:NWAY, (NETAIL - 1) * PAIRCOLS : NETAIL * PAIRCOLS],
                e_t[NU - 1][0:NWAY, :],
            )
            nc.scalar.dma_start(
                etail[NWAY:128, (NETAIL - 1) * PAIRCOLS : NETAIL * PAIRCOLS],
                e_t[NU - 1][NWAY:128, :],
            )
            for t in range(NETAIL - 1):
                u = NU - NETAIL + t
                nc.sync.dma_start(
                    etail[:, t * PAIRCOLS : (t + 1) * PAIRCOLS],
                    e_t[u][:],
                )
            nc.sync.dma_start(rsum[:], r_sb[:])

    n = _dedup_ldweights(nc)
    if n < 64:
        print(f"[kernel] warning: ldweights dedup removed only {n}", flush=True)
    nc.compile()
    return nc


def _get_nc():
    if "nc" not in _CACHE:
        _CACHE["nc"] = _build_nc()
    return _CACHE["nc"]


def _pack_core_q(qc8):
    # fp8 [64, C, F2] -> [p, pair, k, i, f] -> [128, NPAIR*QCOLS]
    qc = qc8.reshape(NPAIR, 2, KT, 128, F2).transpose(3, 0, 2, 1, 4)
    return np.ascontiguousarray(qc).reshape(128, NPAIR * QCOLS)


def _prepare(query_features, labels, prototypes, indices):
    """Returns (in_maps, labels_i64, pn64)."""
    qf = np.asarray(query_features, dtype=np.float32).reshape(B, C, F2)
    labels = np.asarray(labels).astype(np.int64)
    protos = np.asarray(prototypes, dtype=np.float32)
    idx = np.asarray(indices).astype(np.int64)

    pg8 = protos[idx].astype(F8_NP)                      # [64, C] fp8
    pg = pg8.astype(np.float64)
    pn64 = np.sum(pg**2, axis=1)                         # matches device G
    negpn2_np = np.ascontiguousarray(
        np.concatenate([-pn64, -pn64]).reshape(128, 1).astype(np.float32)
    )
    pT_pack = np.ascontiguousarray(
        pg8.T.reshape(KT, 128, NWAY).transpose(1, 0, 2)
    ).reshape(128, KT * NWAY)
    bsel2_np = np.zeros((128, 2 * NPAIR - 1), dtype=BF16_NP)
    bsel2_np[0:NWAY, NPAIR - 1] = 1
    bsel2_np[NWAY:128, NPAIR] = 1

    qf8 = qf.astype(F8_NP)
    in_maps = [
        {
            "q": _pack_core_q(qf8[c * BPC : (c + 1) * BPC]),
            "pT": pT_pack,
            "negpn2": negpn2_np,
            "bsel2": bsel2_np,
        }
        for c in range(NCORES)
    ]
    return in_maps, labels, pn64


def kernel(query_features, labels, prototypes, indices, n_way):
    import time as _time

    t0 = _time.time()
    nc = _get_nc()
    t1 = _time.time()
    in_maps, labels, pn64 = _prepare(query_features, labels, prototypes, indices)
    t2 = _time.time()
    results = run_bass_kernel_spmd(nc, in_maps, list(range(NCORES))).results
    t3 = _time.time()
    print(
        f"[kernel] build={t1 - t0:.1f}s pack={t2 - t1:.1f}s run={t3 - t2:.1f}s",
        flush=True,
    )

    # Host-side finish: rsum[:, 0:26] holds per-image rowsums of G for units
    # 0..12; image local index l lives at row block 64*(l%4>=2)+class,
    # column 2*(l//4)+(l%2).  rsum[0:26, 26] holds the chain-A per-pair-row
    # sums of log s.  Units 13..15 ship raw e = exp(2G - pn) in etail: the
    # host does both their LSE (colsum+log) and their label terms, using
    # log e[label_row] = 2*G[label_row] - pn[label] summed over the image's
    # pixel columns — exactly the per-image label term.
    NRED = 2 * (CHAIN_A_LAST + 1)
    ndev = NRED * 2                                      # images via device rowsums
    larr = np.arange(ndev)
    rows0 = 64 * ((larr % 4) >= 2)
    cols = 2 * (larr // 4) + (larr % 2)
    total_lse = 0.0
    label_term = 0.0
    for c in range(NCORES):
        out = results[c]["rsum"].astype(np.float64)      # [128, 27]
        total_lse += float(out[0:NRED, NRED].sum())
        et = results[c]["etail"].astype(np.float64)      # [128, 3*392]
        total_lse += float(
            np.log(et[0:NWAY].sum(axis=0)).sum()
            + np.log(et[NWAY:128].sum(axis=0)).sum()
        )
        lab = labels[c * BPC : (c + 1) * BPC]
        r2 = out[:, 0:NRED]
        label_term += float(
            np.sum(2.0 * r2[rows0 + lab[:ndev], cols] - F2 * pn64[lab[:ndev]])
        )
        for l in range(ndev, BPC):
            u = l // 4
            blk = et[:, (u - (NU - NETAIL)) * PAIRCOLS + (l % 2) * F2 :
                     (u - (NU - NETAIL)) * PAIRCOLS + (l % 2) * F2 + F2]
            row = 64 * ((l % 4) >= 2) + lab[l]
            label_term += float(np.log(blk[row]).sum())
    loss = (total_lse - label_term) / (B * F2)
    return np.asarray(loss, dtype=np.float32)

